# revision 18
# baseline (speedup 1.0000x reference)
"""Trainium2 Bass kernel for nn_Attention_49185965473844.

Math (per example b):
    q = x @ Wq ; k = x @ Wk ; v = x @ Wv          (x: [S, D], W*: [D, D], D=32)
    A[q,k]   = sum_s q[s,q] k[s,k]  = (Wq^T G Wk)[q,k],   G = x^T x   ([32, 32])
    scores   = softmax(A, axis=q)                 (normalize down columns)
    out[q,s] = sum_k scores[q,k] v[s,k] = (M @ x^T)[q,s], M = scores @ Wv^T

So the whole problem reduces to: one Gram matrix G = x^T x per example (the
only big contraction), a tiny 32x32 chain + softmax, and one [32,32] @ [32,S]
matmul against x^T (PE transposes of the resident x tile).

The kernel is HBM/DMA-bound (16 MB of unavoidable traffic per core), so the
SBUF layout of x is chosen to give large contiguous DMA descriptors on BOTH
the load and the store:

    s = 2048*c + 16*p + l,   c in [0,4), p in [0,128) (partition), l in [0,16)

  * load:  nat[p, (c,l,d)] = x[s,d]  -> per partition 4 runs of 16 rows
    = 2 KB contiguous each (vs 128 B in a plain "(c p) d" layout).
  * an ACT/DVE copy reorders+casts to natP[p, (l,c,d)] in bf16 (the PE
    moving/stationary operands need single-stride APs; bf16 keeps every PE
    op at 1 cyc/row and halves copy cost; measured end-to-end rel err ~2e-3
    vs the 2e-2 gate).
  * PE transpose of the [128, (c d)] block at fixed l gives
    T_l[(c,d), p] = x^T with partition group c = the TOP 2 bits of s.
  * block-diag matmul (bd columns ordered (q, g)) -> o[(q,c), p].
  * the mandatory PSUM->SBUF copy scatters columns p -> 16*p + l, so the
    assembled O_sb[(q,c), f] = out[q, 2048*c + f] stores as ONE fully
    contiguous 1 MB DMA per example.

The per-example work is software-pipelined so the PE never idles (HAM stays
at 2.4 GHz): iteration b runs gram+transposes of example b, the 32x32
chain/softmax of example b-1, and the output matmuls/stores of example b-2.

Sharding: pure data parallel over batch B=64 -> 8 examples per NeuronCore.
"""

import numpy as np
import ml_dtypes

import concourse.bass as bass
import concourse.bacc as bacc
import concourse.tile as tile
from concourse import mybir
from concourse.bass_utils import run_bass_kernel_spmd

N_CORES = 8
B, S, D = 64, 8192, 32
PER_CORE = B // N_CORES  # 8

F32 = mybir.dt.float32
FP16 = mybir.dt.float16

# numpy dtype of the "eye" input fed to the kernel (used by test harnesses)
_EYE_NP_DTYPE = np.float16

N_C = 4    # s bits 11..12: partition group of the transposed tiles
N_L = 16   # s bits 0..3:  within-partition interleave (load run = 16 rows)
N_P = 128  # s bits 4..10: SBUF partition of the natural tile


def build_nc(n_ex=PER_CORE, seq=S):
    """Build the per-core Bass program. Same program runs on all 8 cores."""
    assert seq == N_C * N_P * N_L
    nc = bacc.Bacc("TRN2", target_bir_lowering=False, debug=False)
    x_t = nc.declare_dram_parameter("x", [n_ex, seq, D], F32, isOutput=False)
    eye_t = nc.declare_dram_parameter("eye", [128, 128], FP16, isOutput=False)
    cst_t = nc.declare_dram_parameter("cst", [128, 352], F32, isOutput=False)
    out_t = nc.declare_dram_parameter("out", [n_ex, D, seq], F32, isOutput=True)

    with tile.TileContext(nc) as tc:
        with (
            tc.tile_pool(name="consts", bufs=1) as consts,
            tc.tile_pool(name="nat_pool", bufs=n_ex) as nat_pool,
            tc.tile_pool(name="natp_pool", bufs=3) as natp_pool,
            tc.tile_pool(name="trhs_pool", bufs=3) as trhs_pool,
            tc.tile_pool(name="osb_pool", bufs=4) as osb_pool,
            tc.tile_pool(name="small_pool", bufs=3) as small_pool,
            tc.tile_pool(name="gram_psum", bufs=2, space="PSUM") as gram_psum,
            tc.tile_pool(name="acc_psum", bufs=2, space="PSUM") as acc_psum,
            tc.tile_pool(name="tp_psum", bufs=2, space="PSUM") as tp_psum,
            tc.tile_pool(name="o_psum", bufs=2, space="PSUM") as o_psum,
        ):
            # ---- constants ----
            cst_sb = consts.tile([128, 352], F32)
            nc.sync.dma_start(out=cst_sb, in_=cst_t[:, :])
            identity = cst_sb[:, 0:128]
            wv4 = cst_sb[:, 128:160]       # np.tile(Wv, (4, 1))
            wq4 = cst_sb[:, 160:192]       # np.tile(Wq, (4, 1))
            wk_sb = cst_sb[0:D, 192:224]
            # qgmask[p, 4*q + g] = 1.0 iff p//32 == g
            qgmask = cst_sb[:, 224:352]
            # Wv replicated on 4 partition blocks, PE-transposed so that
            # wvt_rep[k, 32*j + d] = Wv[d, k].
            wvt_ps = acc_psum.tile([D, 128], F32, tag="acc")
            nc.tensor.transpose(wvt_ps, wv4, identity)
            wvt_rep = consts.tile([D, 128], F32)
            nc.scalar.copy(out=wvt_rep, in_=wvt_ps)
            ident_b = consts.tile([128, 128], FP16)
            nc.sync.dma_start(out=ident_b, in_=eye_t[:, :])

            def load_nat(b):
                # nat[p, c, l, d] = x[b, 2048c + 16p + l, d] cast fp32->fp16
                # in the DMA (SWDGE); per partition the (l, d) block is 16
                # rows = 2 KB contiguous in DRAM. The gpsimd DMA instruction
                # issues in ~0.7us vs ~1.5us on the HWDGE rings, so the 8
                # loads start streaming almost immediately.
                nat = nat_pool.tile([128, N_C, N_L, D], FP16, tag="nat",
                                    name=f"nat_{b}")
                nc.gpsimd.dma_start(
                    out=nat,
                    in_=x_t[b].rearrange("(c p l) d -> p c l d",
                                         c=N_C, p=N_P, l=N_L),
                )
                return nat

            # All example loads are queued upfront (x is SBUF-resident for
            # the whole kernel) on the gpsimd SWDGE queue; stores ride the
            # sync HWDGE queue so load and store packets interleave at the
            # DMA engines.
            nats = [load_nat(b) for b in range(n_ex)]

            def make_reorder(b):
                """(c,l,d) -> (l,c,d), fp16->fp16, split ACT/DVE. Only the
                transposes need this order; the gram reads nat directly."""
                natP = natp_pool.tile([128, N_L, N_C, D], FP16, tag="natp",
                                      name=f"natp_{b}")
                src = nats[b].rearrange("p c l d -> p l c d")
                h = N_L // 2
                nc.vector.tensor_copy(out=natP[:, 0:h], in_=src[:, 0:h])
                nc.scalar.copy(out=natP[:, h:N_L], in_=src[:, h:N_L])
                return natP

            # per-example state carried across pipeline stages
            st = [dict() for _ in range(n_ex)]
            natPs = {0: make_reorder(0)}

            def make_tp(b, t):
                """T[(c,d), p] = x[2048c + 16p + (4t+i), d], i in 0..4."""
                natP2 = st[b]["natP2"]
                tp_ps = tp_psum.tile([128, 512], FP16, tag="tp",
                                     name=f"tp_{b}_{t}")
                for i in range(4):
                    l0 = 4 * t + i
                    nc.tensor.transpose(
                        tp_ps[:, 128 * i:128 * (i + 1)],
                        natP2[:, 128 * l0:128 * (l0 + 1)],
                        ident_b,
                    )
                st[b][f"tp{t}"] = tp_ps

            def copy_trhs(b, t):
                """PSUM->SBUF copy of transpose batch t into the [128,2048]
                fp16 rhs tile for the output matmuls (iteration b+2)."""
                if t == 0:
                    st[b]["trhs"] = trhs_pool.tile(
                        [128, 2048], FP16, tag="trhs", name=f"trhs_{b}"
                    )
                dst = st[b]["trhs"][:, 512 * t:512 * (t + 1)]
                nc.vector.tensor_copy(out=dst, in_=st[b][f"tp{t}"])
                st[b].pop(f"tp{t}")

            def out_mm(b2, t):
                """One output matmul o = bd @ trhs[:, 512t:] for example
                b-2, PSUM->SBUF shuffle copy emitted separately."""
                s2 = st[b2]
                o_ps = o_psum.tile([128, 512], F32, tag="o")
                nc.tensor.matmul(
                    o_ps, lhsT=s2["bd"],
                    rhs=s2["trhs"][:, 512 * t:512 * (t + 1)],
                )
                s2[f"o_ps{t}"] = o_ps

            def out_copy(b2, t, eng):
                """o_ps[z, 128i + p] -> o_sb[z, p, 4t + i]."""
                s2 = st[b2]
                o_ps = s2.pop(f"o_ps{t}")
                dst = s2["o_sb"][:, :, 4 * t:4 * (t + 1)]
                src = o_ps.rearrange("z (i p) -> z p i", i=4)
                if eng == "v":
                    nc.vector.tensor_copy(out=dst, in_=src)
                else:
                    nc.scalar.copy(out=dst, in_=src)

            for it in range(n_ex + 2):
                b = it            # gram/transpose stage
                b1 = it - 1       # chain/softmax stage
                b2 = it - 2       # output-matmul/store stage
                in_b = 0 <= b < n_ex
                in_b1 = 0 <= b1 < n_ex
                in_b2 = 0 <= b2 < n_ex
                s1 = st[b1] if in_b1 else None
                s2 = st[b2] if in_b2 else None

                # ---- PE: gram(b): 16 accumulating fp16 [128,128] self
                # products straight from the cast-loaded tile (any column
                # grouping works); diagonal 32x32 blocks sum to G ----
                if in_b:
                    natP = natPs.pop(b)
                    st[b]["natP2"] = natP.rearrange("p l c d -> p (l c d)")
                    nat2 = nats[b].rearrange("p c l d -> p (c l d)")
                    gram_ps = gram_psum.tile([128, 128], F32, tag="gram",
                                             name=f"gram_{b}")
                    n_blk = (N_C * N_L * D) // 128  # 16
                    for t in range(n_blk):
                        nc.tensor.matmul(
                            gram_ps,
                            lhsT=nat2[:, 128 * t:128 * (t + 1)],
                            rhs=nat2[:, 128 * t:128 * (t + 1)],
                            start=(t == 0),
                            stop=(t == n_blk - 1),
                        )
                    st[b]["gram_ps"] = gram_ps

                # ---- chain(b1): fold -> G -> t2 = G @ Wq ----
                if in_b1:
                    gsb = s1["gram_sb"]
                    g_ps = acc_psum.tile([D, D], F32, tag="acc")
                    for j in range(4):
                        nc.tensor.matmul(
                            g_ps,
                            lhsT=identity[:, 32 * j:32 * (j + 1)],
                            rhs=gsb[:, 32 * j:32 * (j + 1)],
                            start=(j == 0),
                            stop=(j == 3),
                        )
                    g_sb = small_pool.tile([D, D], F32, tag="g_sb")
                    nc.scalar.copy(out=g_sb, in_=g_ps)
                    t2_ps = acc_psum.tile([D, D], F32, tag="acc")
                    nc.tensor.matmul(t2_ps, lhsT=g_sb, rhs=wq4[0:D, :])
                    t2_sb = small_pool.tile([D, D], F32, tag="t2_sb")
                    nc.scalar.copy(out=t2_sb, in_=t2_ps)

                if in_b:
                    make_tp(b, 0)
                    copy_trhs(b, 0)

                # ---- chain(b1): A^T and softmax (DVE/ACT get these right
                # after their tiny chain copies, ahead of the bulk) ----
                if in_b1:
                    at_ps = acc_psum.tile([D, D], F32, tag="acc")
                    nc.tensor.matmul(at_ps, lhsT=wk_sb, rhs=t2_sb)
                    nmax = small_pool.tile([D, 1], F32, tag="nmax")
                    nc.vector.reduce_max(
                        out=nmax, in_=at_ps, axis=mybir.AxisListType.X,
                        negate=True,
                    )
                    e_sb = small_pool.tile([D, D], F32, tag="e_sb")
                    rsum = small_pool.tile([D, 1], F32, tag="rsum")
                    nc.scalar.activation(
                        out=e_sb, in_=at_ps,
                        func=mybir.ActivationFunctionType.Exp,
                        bias=nmax, scale=1.0,
                        accum_out=rsum,
                    )
                    rinv = small_pool.tile([D, 1], F32, tag="rinv")
                    nc.vector.reciprocal(out=rinv, in_=rsum)
                    sc_sb = small_pool.tile([D, D], F32, tag="sc_sb")
                    nc.vector.tensor_scalar_mul(out=sc_sb, in0=e_sb,
                                                scalar1=rinv)

                if in_b:
                    make_tp(b, 1)

                if in_b2:
                    s2["o_sb"] = osb_pool.tile(
                        [128, N_P, N_L], F32, tag="o_sb", name=f"osb_{b2}"
                    )
                    out_mm(b2, 0)
                    out_mm(b2, 1)
                    copy_trhs2 = None  # placeholder to keep order explicit

                if in_b:
                    copy_trhs(b, 1)

                if in_b2:
                    out_copy(b2, 0, "v")

                if in_b:
                    make_tp(b, 2)

                # ---- M^T(b1) + bd mask-mul on gpsimd ----
                if in_b1:
                    m4_ps = acc_psum.tile([128, D], F32, tag="acc")
                    nc.tensor.matmul(m4_ps, lhsT=wvt_rep, rhs=sc_sb)
                    m4_sb = small_pool.tile([128, D], F32, tag="m4_sb")
                    nc.scalar.copy(out=m4_sb, in_=m4_ps)
                    bd = small_pool.tile([128, 128], FP16, tag="bd")
                    m4_bcast = bass.AP(
                        tensor=m4_sb.tensor,
                        offset=m4_sb.offset,
                        ap=[list(m4_sb.ap[0]), list(m4_sb.ap[1]), [0, 4]],
                    )
                    nc.gpsimd.tensor_mul(
                        out=bd.rearrange("p (q g) -> p q g", g=4),
                        in0=m4_bcast,
                        in1=qgmask.rearrange("p (q g) -> p q g", g=4),
                    )
                    s1["bd"] = bd

                # gram fold copy of (b): after the chain copies on ACT so
                # it never delays them; feeds iteration b+1's fold
                if in_b:
                    gram_sb = small_pool.tile([128, 128], F32, tag="gram_sb")
                    nc.scalar.copy(out=gram_sb, in_=st[b]["gram_ps"])
                    st[b]["gram_sb"] = gram_sb

                if in_b2:
                    out_mm(b2, 2)

                if in_b:
                    copy_trhs(b, 2)

                if in_b2:
                    out_copy(b2, 1, "s")
                    out_mm(b2, 3)
                    out_copy(b2, 2, "s")

                if in_b:
                    make_tp(b, 3)
                    copy_trhs(b, 3)

                if in_b2:
                    out_copy(b2, 3, "s")
                    # store: one fully contiguous 1 MB DMA per example on
                    # the otherwise-idle sync HWDGE queue
                    nc.sync.dma_start(
                        out=out_t[b2].rearrange("q (c f) -> (q c) f", c=N_C),
                        in_=s2["o_sb"].rearrange("z p l -> z (p l)"),
                    )

                # prefetch: reorder of example b+1 on DVE/ACT (queue tails)
                if 0 <= b + 1 < n_ex:
                    natPs[b + 1] = make_reorder(b + 1)

    nc.compile()
    return nc


_CACHED_NC = None


def _get_nc():
    global _CACHED_NC
    if _CACHED_NC is None:
        _CACHED_NC = build_nc()
    return _CACHED_NC


def make_cst(wq, wk, wv):
    """[128, 352]: identity | tile(Wv,(4,1)) | Wq | Wk | (q,g) group mask."""
    cst = np.zeros((128, 352), dtype=np.float32)
    cst[:, 0:128] = np.eye(128, dtype=np.float32)
    cst[:, 128:160] = np.tile(wv, (4, 1))
    cst[:, 160:192] = np.tile(wq, (4, 1))
    cst[0:D, 192:224] = wk
    pblk = np.arange(128) // 32
    g = np.arange(128) % 4
    cst[:, 224:352] = (pblk[:, None] == g[None, :]).astype(np.float32)
    return cst


def kernel(x, Wq, Wk, Wv):
    x = np.ascontiguousarray(np.asarray(x, dtype=np.float32))
    wq = np.asarray(Wq, dtype=np.float32).reshape(D, D)
    wk = np.asarray(Wk, dtype=np.float32).reshape(D, D)
    wv = np.asarray(Wv, dtype=np.float32).reshape(D, D)
    assert x.shape == (B, S, D)
    cst = make_cst(wq, wk, wv)

    nc = _get_nc()
    eye = np.eye(128, dtype=ml_dtypes.bfloat16)
    in_maps = [
        {
            "x": x[c * PER_CORE:(c + 1) * PER_CORE],
            "cst": cst,
            "eye": eye,
        }
        for c in range(N_CORES)
    ]
    res = run_bass_kernel_spmd(nc, in_maps, list(range(N_CORES)))
    out = np.concatenate([res.results[c]["out"] for c in range(N_CORES)], axis=0)
    return out


# revision 19
# speedup vs baseline: 1.0185x; 1.0185x over previous
"""Trainium2 Bass kernel for nn_Attention_49185965473844.

Math (per example b):
    q = x @ Wq ; k = x @ Wk ; v = x @ Wv          (x: [S, D], W*: [D, D], D=32)
    A[q,k]   = sum_s q[s,q] k[s,k]  = (Wq^T G Wk)[q,k],   G = x^T x   ([32, 32])
    scores   = softmax(A, axis=1)                 (normalize down columns)
    out[q,s] = sum_k scores[q,k] v[s,k] = (M @ x^T)[q,s], M = scores @ Wv^T

So the whole problem reduces to: one Gram matrix G = x^T x per example, a
tiny 32x32 chain + softmax, and one [32,32] @ [32,S] matmul against x^T.

The kernel is HBM/DMA-bound (16 MB of unavoidable traffic per core), so the
layout is designed around the DMA and the DVE's 32x32 block transpose:

    s = 2048*g + 64*p' + j,  g in [0,4), p' in [0,32), j in [0,64)
    SBUF partition p = 32*g + p' (the TOP 7 bits of s)

  * load: nat[p, (r=j, d)] = x[64p + j, d] is x's natural row-major order:
    fully contiguous 8 KB per partition, cast fp32->fp16 in the DMA (SWDGE).
    fp16 (10-bit mantissa) keeps every PE matmul at 1 cyc/row with FWL
    weight loads; measured end-to-end rel err 7.8e-4 vs the 2e-2 gate.
  * gram: 16 accumulating fp16 [128,128] self products of column blocks;
    the diagonal 32x32 blocks sum to G.
  * the DVE 32x32 block transpose of nat IS the output-matmul rhs:
    T[(g,k), 32j + p'] = x[2048g + 64p' + j, k] - partition group g is the
    top 2 bits of s, so one SBUF->SBUF DVE op replaces all PE transposes.
  * block-diag matmul (bd columns ordered (q, g)) -> o[(q,g), (j, p')].
  * the mandatory PSUM->SBUF copy scatters (j, p') -> 64p' + j, so the
    assembled o_sb[(q,g), f] = out[q, 2048g + f] stores as ONE fully
    contiguous 1 MB DMA per example (on the otherwise idle sync queue).

The per-example work is software-pipelined so the PE never idles long
(HAM stays at 2.4 GHz): iteration i runs gram+transpose of example i, the
chain/softmax of example i-1, and the output matmuls/store of example i-2.

Sharding: pure data parallel over batch B=64 -> 8 examples per NeuronCore.
"""

import numpy as np

import concourse.bass as bass
import concourse.bacc as bacc
import concourse.tile as tile
from concourse import mybir
from concourse.bass_utils import run_bass_kernel_spmd

N_CORES = 8
B, S, D = 64, 8192, 32
PER_CORE = B // N_CORES  # 8

F32 = mybir.dt.float32
FP16 = mybir.dt.float16

N_R = 64   # s bits 0..5: rows per partition (load run = 64 rows = 8 KB)
N_P = 128  # s bits 6..12: SBUF partition


def build_nc(n_ex=PER_CORE, seq=S):
    """Build the per-core Bass program. Same program runs on all 8 cores."""
    assert seq == N_P * N_R
    nc = bacc.Bacc("TRN2", target_bir_lowering=False, debug=False)
    x_t = nc.declare_dram_parameter("x", [n_ex, seq, D], F32, isOutput=False)
    cst_t = nc.declare_dram_parameter("cst", [128, 352], F32, isOutput=False)
    out_t = nc.declare_dram_parameter("out", [n_ex, D, seq], F32, isOutput=True)

    with tile.TileContext(nc) as tc:
        with (
            tc.tile_pool(name="consts", bufs=1) as consts,
            tc.tile_pool(name="nat_pool", bufs=n_ex) as nat_pool,
            tc.tile_pool(name="trhs_pool", bufs=3) as trhs_pool,
            tc.tile_pool(name="osb_pool", bufs=4) as osb_pool,
            tc.tile_pool(name="small_pool", bufs=3) as small_pool,
            tc.tile_pool(name="gram_psum", bufs=2, space="PSUM") as gram_psum,
            tc.tile_pool(name="acc_psum", bufs=2, space="PSUM") as acc_psum,
            tc.tile_pool(name="o_psum", bufs=3, space="PSUM") as o_psum,
        ):
            # ---- constants ----
            cst_sb = consts.tile([128, 352], F32)
            nc.sync.dma_start(out=cst_sb, in_=cst_t[:, :])
            identity = cst_sb[:, 0:128]
            wv4 = cst_sb[:, 128:160]       # np.tile(Wv, (4, 1))
            wq4 = cst_sb[:, 160:192]       # np.tile(Wq, (4, 1))
            wk_sb = cst_sb[0:D, 192:224]
            # qgmask[p, 4*q + g] = 1.0 iff p//32 == g
            qgmask = cst_sb[:, 224:352]
            # Wv replicated on 4 partition blocks, PE-transposed so that
            # wvt_rep[k, 32*j + d] = Wv[d, k].
            wvt_ps = acc_psum.tile([D, 128], F32, tag="acc")
            nc.tensor.transpose(wvt_ps, wv4, identity)
            wvt_rep = consts.tile([D, 128], F32)
            nc.scalar.copy(out=wvt_rep, in_=wvt_ps)

            def load_nat(b):
                # nat[p, r, d] = x[b, 64p + r, d] cast fp32->fp16 in the
                # DMA (SWDGE): per partition one fully contiguous 8 KB read.
                nat = nat_pool.tile([128, N_R, D], FP16, tag="nat",
                                    name=f"nat_{b}")
                nc.gpsimd.dma_start(
                    out=nat,
                    in_=x_t[b].rearrange("(p r) d -> p r d", p=N_P, r=N_R),
                )
                return nat

            # All example loads are queued upfront (x is SBUF-resident for
            # the whole kernel) on the gpsimd SWDGE queue; stores ride the
            # sync HWDGE queue so load and store packets interleave at the
            # DMA engines.
            nats = [load_nat(b) for b in range(n_ex)]

            # per-example state carried across pipeline stages
            st = [dict() for _ in range(n_ex)]

            def out_mm(b2, t):
                """One output matmul o = bd @ trhs[:, 512t:] for example
                b-2; the PSUM->SBUF shuffle copy is emitted separately."""
                s2 = st[b2]
                o_ps = o_psum.tile([128, 512], F32, tag="o")
                nc.tensor.matmul(
                    o_ps, lhsT=s2["bd"],
                    rhs=s2["trhs"][:, 512 * t:512 * (t + 1)],
                )
                s2[f"o_ps{t}"] = o_ps

            def out_copy(b2, t, eng):
                """o_ps[z, 32 j2 + p'] -> o_sb[z, p', 16t + j2]."""
                s2 = st[b2]
                o_ps = s2.pop(f"o_ps{t}")
                dst = s2["o_sb"][:, :, 16 * t:16 * (t + 1)]
                src = o_ps.rearrange("z (j p) -> z p j", j=16, p=32)
                if eng == "v":
                    nc.vector.tensor_copy(out=dst, in_=src)
                else:
                    nc.scalar.copy(out=dst, in_=src)

            n_blk = (N_R * D) // 128  # 16 gram column blocks

            for it in range(n_ex + 2):
                b = it            # gram/transpose stage
                b1 = it - 1       # chain/softmax stage
                b2 = it - 2       # output-matmul/store stage
                in_b = 0 <= b < n_ex
                in_b1 = 0 <= b1 < n_ex
                in_b2 = 0 <= b2 < n_ex
                s1 = st[b1] if in_b1 else None
                s2 = st[b2] if in_b2 else None

                # ---- PE: gram(b) part 1: fp16 [128,128] self products of
                # column blocks; diagonal 32x32 blocks sum to G ----
                if in_b:
                    nat2 = nats[b].rearrange("p r d -> p (r d)")
                    st[b]["nat2"] = nat2
                    gram_ps = gram_psum.tile([128, 128], F32, tag="gram",
                                             name=f"gram_{b}")
                    st[b]["gram_ps"] = gram_ps
                    for t in range(10):
                        nc.tensor.matmul(
                            gram_ps,
                            lhsT=nat2[:, 128 * t:128 * (t + 1)],
                            rhs=nat2[:, 128 * t:128 * (t + 1)],
                            start=(t == 0),
                            stop=False,
                            skip_group_check=True,
                        )

                # ---- DVE: block transpose half A of (b): SBUF->SBUF,
                # directly produces the output-matmul rhs tile ----
                if in_b:
                    trhs = trhs_pool.tile([128, 2048], FP16, tag="trhs",
                                          name=f"trhs_{b}")
                    st[b]["trhs"] = trhs
                    nc.vector.transpose(out=trhs[:, 0:1024],
                                        in_=st[b]["nat2"][:, 0:1024])

                # ---- chain(b1): fold -> G -> t2 = G @ Wq ----
                if in_b1:
                    gsb = s1["gram_sb"]
                    g_ps = acc_psum.tile([D, D], F32, tag="acc")
                    for j in range(4):
                        nc.tensor.matmul(
                            g_ps,
                            lhsT=identity[:, 32 * j:32 * (j + 1)],
                            rhs=gsb[:, 32 * j:32 * (j + 1)],
                            start=(j == 0),
                            stop=(j == 3),
                        )
                    g_sb = small_pool.tile([D, D], F32, tag="g_sb")
                    nc.scalar.copy(out=g_sb, in_=g_ps)
                    t2_ps = acc_psum.tile([D, D], F32, tag="acc")
                    nc.tensor.matmul(t2_ps, lhsT=g_sb, rhs=wq4[0:D, :])
                    t2_sb = small_pool.tile([D, D], F32, tag="t2_sb")
                    nc.scalar.copy(out=t2_sb, in_=t2_ps)

                if in_b2:
                    s2["o_sb"] = osb_pool.tile(
                        [128, 32, N_R], F32, tag="o_sb", name=f"osb_{b2}"
                    )
                    out_mm(b2, 0)

                # ---- chain(b1): A^T and softmax ----
                if in_b1:
                    at_ps = acc_psum.tile([D, D], F32, tag="acc")
                    nc.tensor.matmul(at_ps, lhsT=wk_sb, rhs=t2_sb)
                    nmax = small_pool.tile([D, 1], F32, tag="nmax")
                    nc.vector.reduce_max(
                        out=nmax, in_=at_ps, axis=mybir.AxisListType.X,
                        negate=True,
                    )
                    e_sb = small_pool.tile([D, D], F32, tag="e_sb")
                    rsum = small_pool.tile([D, 1], F32, tag="rsum")
                    nc.scalar.activation(
                        out=e_sb, in_=at_ps,
                        func=mybir.ActivationFunctionType.Exp,
                        bias=nmax, scale=1.0,
                        accum_out=rsum,
                    )
                    rinv = small_pool.tile([D, 1], F32, tag="rinv")
                    nc.vector.reciprocal(out=rinv, in_=rsum)
                    sc_sb = small_pool.tile([D, D], F32, tag="sc_sb")
                    nc.vector.tensor_scalar_mul(out=sc_sb, in0=e_sb,
                                                scalar1=rinv)

                # ---- PE: gram(b) part 2 (fills the softmax latency) ----
                if in_b:
                    gram_ps = st[b]["gram_ps"]
                    nat2 = st[b]["nat2"]
                    for t in range(10, n_blk):
                        nc.tensor.matmul(
                            gram_ps,
                            lhsT=nat2[:, 128 * t:128 * (t + 1)],
                            rhs=nat2[:, 128 * t:128 * (t + 1)],
                            start=False,
                            stop=(t == n_blk - 1),
                            skip_group_check=True,
                        )

                if in_b2:
                    out_mm(b2, 1)
                    out_copy(b2, 0, "v")
                    out_mm(b2, 2)

                # ---- PE: M^T(b1) + bd mask-mul on gpsimd ----
                if in_b1:
                    m4_ps = acc_psum.tile([128, D], F32, tag="acc")
                    nc.tensor.matmul(m4_ps, lhsT=wvt_rep, rhs=sc_sb)
                    m4_sb = small_pool.tile([128, D], F32, tag="m4_sb")
                    nc.scalar.copy(out=m4_sb, in_=m4_ps)
                    # Block-diagonal lhsT for the output matmuls, columns
                    # ordered (q, g) so the output partition 4q + g is
                    # affine in the DRAM row of out[b]. The mask multiply
                    # also casts to fp16.
                    bd = small_pool.tile([128, 128], FP16, tag="bd")
                    m4_bcast = bass.AP(
                        tensor=m4_sb.tensor,
                        offset=m4_sb.offset,
                        ap=[list(m4_sb.ap[0]), list(m4_sb.ap[1]), [0, 4]],
                    )
                    nc.gpsimd.tensor_mul(
                        out=bd.rearrange("p (q g) -> p q g", g=4),
                        in0=m4_bcast,
                        in1=qgmask.rearrange("p (q g) -> p q g", g=4),
                    )
                    s1["bd"] = bd

                # gram fold copy of (b): after the chain copies on ACT so
                # it never delays them; feeds iteration b+1's fold
                if in_b:
                    gram_sb = small_pool.tile([128, 128], F32, tag="gram_sb")
                    nc.scalar.copy(out=gram_sb, in_=st[b]["gram_ps"])
                    st[b]["gram_sb"] = gram_sb

                if in_b2:
                    out_mm(b2, 3)
                    out_copy(b2, 1, "s")

                # ---- DVE: block transpose half B of (b) ----
                if in_b:
                    nc.vector.transpose(out=st[b]["trhs"][:, 1024:2048],
                                        in_=st[b]["nat2"][:, 1024:2048])

                if in_b2:
                    out_copy(b2, 2, "s")
                    out_copy(b2, 3, "s")
                    # store: one fully contiguous 1 MB DMA per example on
                    # the otherwise-idle sync HWDGE queue
                    nc.sync.dma_start(
                        out=out_t[b2].rearrange("q (c f) -> (q c) f", c=4),
                        in_=s2["o_sb"].rearrange("z p l -> z (p l)"),
                    )

    nc.compile()
    return nc


_CACHED_NC = None


def _get_nc():
    global _CACHED_NC
    if _CACHED_NC is None:
        _CACHED_NC = build_nc()
    return _CACHED_NC


def make_cst(wq, wk, wv):
    """[128, 352]: identity | tile(Wv,(4,1)) | tile(Wq,(4,1)) | Wk | mask."""
    cst = np.zeros((128, 352), dtype=np.float32)
    cst[:, 0:128] = np.eye(128, dtype=np.float32)
    cst[:, 128:160] = np.tile(wv, (4, 1))
    cst[:, 160:192] = np.tile(wq, (4, 1))
    cst[0:D, 192:224] = wk
    pblk = np.arange(128) // 32
    g = np.arange(128) % 4
    cst[:, 224:352] = (pblk[:, None] == g[None, :]).astype(np.float32)
    return cst


def kernel(x, Wq, Wk, Wv):
    x = np.ascontiguousarray(np.asarray(x, dtype=np.float32))
    wq = np.asarray(Wq, dtype=np.float32).reshape(D, D)
    wk = np.asarray(Wk, dtype=np.float32).reshape(D, D)
    wv = np.asarray(Wv, dtype=np.float32).reshape(D, D)
    assert x.shape == (B, S, D)
    cst = make_cst(wq, wk, wv)

    nc = _get_nc()
    in_maps = [
        {
            "x": x[c * PER_CORE:(c + 1) * PER_CORE],
            "cst": cst,
        }
        for c in range(N_CORES)
    ]
    res = run_bass_kernel_spmd(nc, in_maps, list(range(N_CORES)))
    out = np.concatenate([res.results[c]["out"] for c in range(N_CORES)], axis=0)
    return out
